# revision 1
# baseline (speedup 1.0000x reference)
"""Trainium2 Bass kernel for nn_Event_Critic_Net (dual-branch GAT critic).

Math: the reference only reads the GAT output at the LAST node of each
graph (graphs are 32 contiguous nodes), so only edges whose dst is a
graph's last node contribute.  For those edges the softmax-weighted
aggregation commutes with the linear projection W:

    out_g = sigmoid( (sum_n alpha[n] * x[n,:]) @ W + bias )
    alpha[n] = cnt[n]*exp(e[n]) / (sum_n cnt[n]*exp(e[n]) + 1e-16)
    e[n] = leaky_relu(x[n]. w_src + x[last(g)]. w_dst),  w_* = W @ att_*

cnt[n] = number of edges (n -> last(g(n))).  Graph-structure prep
(edge counts, tiling, transposed copy, weight replication) happens on
host; all FLOPs on device.  Sharding: graphs are data-parallel across
the 8 cores (core c owns graphs [c*512,(c+1)*512)).

x is shipped twice in bf16: node-major (y aggregation, PE contracts
over nodes) and s-major `xt` (attention logits, PE contracts over
features).  PSUM accumulates fp32; softmax scalars stay fp32.
"""

import numpy as np
from contextlib import ExitStack

NC = 8            # cores
N = 131072        # nodes total
G = 4096          # graphs
NPG = 32          # nodes per graph
S = 64            # state size
H = 128           # hidden size
NPC = N // NC     # 16384 nodes per core
GPC = G // NC     # 512 graphs per core
T = NPC // 128    # 128 node-tiles per core
SA = 66           # x columns: 64 features | ones@64 | zero pad
TH = T // 2       # half-branch tiles

_CACHE = {}


def _build_module():
    import concourse.tile as tile
    from concourse import bacc, mybir
    from concourse.alu_op_type import AluOpType as Alu

    f32 = mybir.dt.float32
    bf16 = mybir.dt.bfloat16
    Act = mybir.ActivationFunctionType
    AxX = mybir.AxisListType.X

    nc = bacc.Bacc("TRN2", target_bir_lowering=False, debug=False,
                   num_devices=NC)

    dram = {}

    def din(name, shape, dt=f32):
        dram[name] = nc.dram_tensor(name, shape, dt, kind="ExternalInput")

    for p in ("u", "d"):
        din(f"{p}_xab", [128, T * SA], bf16)
        din(f"{p}_xt", [128, NPC // 2], bf16)
        din(f"{p}_cnt", [128, T])
        din(f"{p}_xlast", [128, 4 * S], bf16)
    din("cstf", [128, 200])
    din("cstb", [128, 912], bf16)
    out_dram = nc.dram_tensor("out", [1, GPC], f32, kind="ExternalOutput")

    with tile.TileContext(nc) as tc, ExitStack() as ctx:
        const = ctx.enter_context(tc.tile_pool(name="const", bufs=1))
        xp = ctx.enter_context(tc.tile_pool(name="xp", bufs=2))
        wk = ctx.enter_context(tc.tile_pool(name="wk", bufs=2))
        ps1 = ctx.enter_context(tc.tile_pool(name="ps1", bufs=1, space="PSUM"))
        ps2 = ctx.enter_context(tc.tile_pool(name="ps2", bufs=2, space="PSUM"))

        cstf = const.tile([128, 200], f32, tag="cstf")
        nc.gpsimd.dma_start(cstf[:], dram["cstf"].ap())
        cstb = const.tile([128, 912], bf16, tag="cstb")
        nc.gpsimd.dma_start(cstb[:], dram["cstb"].ap())
        Bm = cstf[:, 0:4]
        eps = cstf[0:1, 4:5]
        mlpb = cstf[0:1, 5:6]
        biases = {"u": cstf[:, 6:7], "d": cstf[:, 7:8]}
        ones64 = cstf[0:1, 8:8 + S]
        ident = cstf[:, 72:200]
        Qm = cstb[0:4, 0:128]
        wv4s = {"u": cstb[:, 128:132], "d": cstb[:, 132:136]}
        wdsts = {"u": cstb[:, 136:392], "d": cstb[:, 392:648]}
        Ws = {"u": cstb[0:64, 648:776], "d": cstb[0:64, 776:904]}
        mlpW = cstb[:, 904:905]

        sig = {}
        st = {"u": {}, "d": {}}
        # ---- big loads: xt then xab; branch u via Sync DGE, d via Scalar ----
        for p, eng in (("u", nc.sync), ("d", nc.scalar)):
            xt2 = []
            for c in range(2):
                t = xp.tile([128, NPC // 4], bf16, tag=f"xt{c}",
                            name=f"xt{c}_{p}")
                eng.dma_start(
                    t[:], dram[f"{p}_xt"].ap()[:, c * NPC // 4:
                                               (c + 1) * NPC // 4])
                xt2.append(t)
            st[p]["xt"] = xt2
            xq = []
            for c in range(2):
                t = xp.tile([128, TH * SA], bf16, tag=f"x{c}",
                            name=f"x{c}_{p}")
                eng.dma_start(
                    t[:], dram[f"{p}_xab"].ap()[:, c * TH * SA:
                                                (c + 1) * TH * SA])
                xq.append(t)
            st[p]["x"] = xq

        # ---- phase A (both branches): small loads + attention logits ----
        for p in ("u", "d"):
            s = st[p]
            wv4 = wv4s[p]
            wdst = wdsts[p]
            s["Wb"] = Ws[p]
            s["bias"] = biases[p]

            cnt = wk.tile([128, T], f32, tag="cnt", name=f"cnt_{p}")
            s["cnt"] = cnt
            nc.gpsimd.dma_start(cnt[:], dram[f"{p}_cnt"].ap())
            xl = wk.tile([128, 4 * S], bf16, tag="xl")
            nc.gpsimd.dma_start(xl[:], dram[f"{p}_xlast"].ap())
            xt2 = s["xt"]

            # a_src per node on PE: one f=4 matmul covers two node-tiles
            # (chunk c: cols 4c+0/1 = tile c, cols 4c+2/3 = tile 64+c)
            asps = ps2.tile([128, 2 * T], f32, tag="asps", name=f"asps_{p}")
            s["asps"] = asps
            for c in range(T // 2):
                xtc = xt2[c // 32]
                cc = c % 32
                nc.tensor.matmul(
                    asps[0:128, 4 * c:4 * c + 4],
                    xtc[:, 128 * cc:128 * cc + 128],
                    wv4,
                    start=True, stop=True)

            # a_dst at last nodes: mult+reduce, transpose, broadcast
            tmp4 = wk.tile([128, 4 * S], bf16, tag="tmp4")
            nc.vector.tensor_tensor(tmp4[:], xl[:], wdst, op=Alu.mult)
            adst = wk.tile([128, 4], f32, tag="adst")
            nc.vector.tensor_reduce(
                adst[:], tmp4[:].rearrange("p (j s) -> p j s", s=S),
                axis=AxX, op=Alu.add)
            tp = ps1.tile([4, 128], f32, tag="mix")
            nc.tensor.transpose(tp[:], adst[:], ident)
            adT = wk.tile([4, 128], bf16, tag="adT")
            nc.vector.tensor_copy(adT[:], tp[:])
            adbc_ps = ps1.tile([128, T], f32, tag="adbc")
            nc.tensor.matmul(adbc_ps[:], Qm, adT[:], start=True, stop=True)
            adbc = wk.tile([128, T], f32, tag="adbcs", name=f"adbcs_{p}")
            s["adbc"] = adbc
            nc.vector.tensor_copy(adbc[:], adbc_ps[:])

        # ---- phase B (both branches): P/M, aggregation, normalize ----
        for p in ("u", "d"):
            s = st[p]
            x, cnt, adbc, asps = s["x"], s["cnt"], s["adbc"], s["asps"]
            M = wk.tile([128, 4 * T], bf16, tag="M")
            Mv = M[:].rearrange("p (i j) -> p i j", j=4)
            for h in range(2):
                hs = slice(h * TH, (h + 1) * TH)
                asrc = wk.tile([128, TH], f32, tag="asrc")
                nc.vector.tensor_copy(asrc[:], asps[:, 2 * h::4])
                z = wk.tile([128, TH], f32, tag="z")
                nc.vector.tensor_tensor(z[:], asrc[:], adbc[:, hs],
                                        op=Alu.add)
                e = wk.tile([128, TH], f32, tag="e")
                nc.vector.scalar_tensor_tensor(
                    e[:], z[:], 0.2, z[:], op0=Alu.mult, op1=Alu.max)
                ex = wk.tile([128, TH], f32, tag="ex")
                nc.scalar.activation(ex[:], e[:], Act.Exp)
                P = wk.tile([128, TH], f32, tag="P")
                nc.vector.tensor_tensor(P[:], ex[:], cnt[:, hs], op=Alu.mult)
                for j in range(4):
                    nc.vector.tensor_scalar(
                        Mv[:, hs, j], P[:], Bm[:, j:j + 1], None, op0=Alu.mult)

            ynT = ps2.tile([128, 4 * T], f32, tag="ynT")
            for i in range(T):
                xc = x[i // TH]
                ii = i % TH
                nc.tensor.matmul(
                    ynT[0:SA, 4 * i:4 * (i + 1)],
                    xc[:, SA * ii:SA * (ii + 1)],
                    M[:, 4 * i:4 * (i + 1)],
                    start=True, stop=True)

            # normalize by denominator (row 64 of y^T)
            ysb = wk.tile([S + 1, GPC], f32, tag="ysb")
            nc.vector.tensor_copy(ysb[:], ynT[0:S + 1, :])
            dn = wk.tile([1, GPC], f32, tag="dn")
            nc.vector.tensor_scalar(
                dn[:], ysb[S:S + 1, :], eps, None, op0=Alu.add)
            rp = wk.tile([1, GPC], f32, tag="rp")
            nc.vector.reciprocal_approx_fast(rp[:], dn[:])
            rbc = ps1.tile([S, GPC], f32, tag="mix")
            nc.tensor.matmul(rbc[:], ones64, rp[:], start=True, stop=True)
            ynrm = wk.tile([S, GPC], bf16, tag="ynrm")
            nc.vector.tensor_tensor(ynrm[:], ysb[0:S, :], rbc[:], op=Alu.mult)

            # project + bias + sigmoid
            hT = ps1.tile([H, GPC], f32, tag="hT")
            nc.tensor.matmul(hT[:], s["Wb"], ynrm[:], start=True, stop=True)
            sg = wk.tile([H, GPC], bf16, tag="sig")
            nc.scalar.activation(sg[:], hT[:], Act.Sigmoid, bias=s["bias"])
            sig[p] = sg

        # ---- combine branches + MLP head ----
        prod = wk.tile([H, GPC], bf16, tag="prod")
        nc.vector.tensor_tensor(prod[:], sig["u"][:], sig["d"][:], op=Alu.mult)
        o_ps = ps1.tile([1, GPC], f32, tag="mix")
        nc.tensor.matmul(o_ps[:], mlpW, prod[:], start=True, stop=True)
        o_sb = wk.tile([1, GPC], f32, tag="o_sb")
        nc.vector.tensor_scalar(
            o_sb[:], o_ps[:], mlpb, None, op0=Alu.add)
        nc.sync.dma_start(out_dram.ap(), o_sb[:])

    nc.compile()
    return nc


def _get_module():
    if "nc" not in _CACHE:
        _CACHE["nc"] = _build_module()
    return _CACHE["nc"]


def _prep_branch(x, ei, W, att_src, att_dst, bias):
    """Host-side sharding + graph-format prep for one branch."""
    import ml_dtypes
    bf = ml_dtypes.bfloat16
    x = np.asarray(x, np.float32)
    src = np.asarray(ei[0]).astype(np.int64)
    dst = np.asarray(ei[1]).astype(np.int64)
    W = np.asarray(W, np.float32)
    w_src = (W @ np.asarray(att_src, np.float32)).astype(np.float32)
    w_dst = (W @ np.asarray(att_dst, np.float32)).astype(np.float32)

    valid = (dst % NPG) == (NPG - 1)
    cnt = np.bincount(src[valid], minlength=N).astype(np.float32)

    per_core = []
    for c in range(NC):
        xs = x[c * NPC:(c + 1) * NPC]
        xab = np.zeros((T, 128, SA), np.float32)
        xab[:, :, :S] = xs.reshape(T, 128, S)
        xab[:, :, S] = 1.0
        xab = np.ascontiguousarray(
            xab.transpose(1, 0, 2).reshape(128, T * SA)).astype(bf)
        # xt[64k+s, m] = x[8192k + m, s]
        xtv = xs.reshape(2, NPC // 2, S).transpose(0, 2, 1)
        xtv = np.ascontiguousarray(xtv.reshape(128, NPC // 2)).astype(bf)
        cnt_t = np.ascontiguousarray(
            cnt[c * NPC:(c + 1) * NPC].reshape(T, 128).T)
        xlast = np.ascontiguousarray(
            xs[NPG - 1::NPG].reshape(128, 4 * S)).astype(bf)
        per_core.append({"xab": xab, "xt": xtv, "cnt": cnt_t, "xlast": xlast})

    wv4 = np.zeros((128, 4), np.float32)
    wv4[:S, 0] = w_src
    wv4[:S, 1] = w_dst
    wv4[S:, 2] = w_src
    wv4[S:, 3] = w_dst
    wdst_rep = np.broadcast_to(w_dst, (128, 4, S)).reshape(128, 4 * S)
    shared = {
        "wv4": wv4.astype(np.float32),
        "wdst": wdst_rep.astype(np.float32),
        "W": W,
        "bias": np.asarray(bias, np.float32).reshape(H, 1),
    }
    return per_core, shared


def _build_in_maps(inputs):
    import ml_dtypes
    bf = ml_dtypes.bfloat16
    pcs = {}
    shareds = {}
    pcs["u"], shareds["u"] = _prep_branch(
        inputs["up_x"], inputs["up_edge_index"], inputs["up_W"],
        inputs["up_att_src"], inputs["up_att_dst"], inputs["up_bias"])
    pcs["d"], shareds["d"] = _prep_branch(
        inputs["down_x"], inputs["down_edge_index"], inputs["down_W"],
        inputs["down_att_src"], inputs["down_att_dst"], inputs["down_bias"])

    pp = np.arange(128)
    cstf = np.zeros((128, 200), np.float32)
    cstf[pp, pp // 32] = 1.0                       # Bm cols 0:4
    cstf[0, 4] = 1e-16                             # eps
    cstf[0, 5] = float(np.asarray(inputs["mlp_b"]).reshape(-1)[0])
    cstf[:, 6] = shareds["u"]["bias"][:, 0]
    cstf[:, 7] = shareds["d"]["bias"][:, 0]
    cstf[0, 8:8 + S] = 1.0                         # ones64
    cstf[:, 72:200] = np.eye(128, dtype=np.float32)

    cstb = np.zeros((128, 912), np.float32)
    cstb[pp // 32, pp] = 0.0
    Qm = np.zeros((4, 128), np.float32)
    Qm[np.arange(128) // 32, np.arange(128)] = 1.0
    cstb[0:4, 0:128] = Qm
    cstb[:, 128:132] = shareds["u"]["wv4"]
    cstb[:, 132:136] = shareds["d"]["wv4"]
    cstb[:, 136:392] = shareds["u"]["wdst"]
    cstb[:, 392:648] = shareds["d"]["wdst"]
    cstb[0:64, 648:776] = shareds["u"]["W"]
    cstb[0:64, 776:904] = shareds["d"]["W"]
    cstb[:, 904] = np.asarray(inputs["mlp_W"], np.float32).reshape(H)

    common = {
        "cstf": cstf,
        "cstb": cstb.astype(bf),
    }

    in_maps = []
    for c in range(NC):
        m = dict(common)
        for p in ("u", "d"):
            for k, v in pcs[p][c].items():
                m[f"{p}_{k}"] = v
        in_maps.append(m)
    return in_maps


def kernel(**inputs):
    from concourse.bass_utils import run_bass_kernel_spmd

    nc = _get_module()
    in_maps = _build_in_maps(inputs)
    res = run_bass_kernel_spmd(nc, in_maps, core_ids=list(range(NC)))
    out = np.concatenate(
        [np.asarray(r["out"], np.float32).reshape(GPC) for r in res.results])
    return out.reshape(G, 1)



# revision 3
# speedup vs baseline: 1.3021x; 1.3021x over previous
"""Trainium2 Bass kernel for nn_Event_Critic_Net (dual-branch GAT critic).

Math: the reference reads the GAT output only at the LAST node of each
graph (graphs are 32 contiguous nodes), so only edges with dst == last
node contribute.  Per graph g:

    out_g = sigmoid( (sum_n alpha[n] x[n]) @ W + bias )
    alpha[n] = cnt[n] exp(e[n]) / (sum + 1e-16)
    e[n] = leaky_relu(x[n].w_src + x[last(g)].w_dst)

Only nodes with cnt>0 matter (~7 of 32 per graph), so the host GATHERS
contributing nodes and bin-packs graphs into 128-slot tiles, T=36 tiles
x C=20 graph-columns per core per branch.  Per-slot logits come from a
single matmul per tile: the feature-major tensor stacks the slot's own
features (rows 0:64) on top of its graph's last-node features (rows
64:128), so lhsT.T @ [w_src; w_dst] yields e directly - no broadcast
chain.  Aggregation contracts slots on the PE with host-built
cnt-masked per-column masks.  Graphs are data-parallel over 8 cores.
"""

import numpy as np
from contextlib import ExitStack

NC = 8            # cores
N = 131072        # nodes total
G = 4096          # graphs
NPG = 32          # nodes per graph
S = 64            # state size
H = 128           # hidden size
GPC = G // NC     # 512 graphs per core
T = 36            # slot tiles per branch per core
C = 20            # graph columns per tile
TC = T * C        # 720 output columns per core
HT = T // 2       # tiles per chunk (2 chunks)
NEG = 0.2

_CACHE = {}


def _build_module():
    import concourse.tile as tile
    from concourse import bacc, mybir
    from concourse.alu_op_type import AluOpType as Alu

    f32 = mybir.dt.float32
    bf16 = mybir.dt.bfloat16
    Act = mybir.ActivationFunctionType

    nc = bacc.Bacc("TRN2", target_bir_lowering=False, debug=False,
                   num_devices=NC)

    dram = {}
    for p in ("u", "d"):
        dram[f"{p}_xgt2"] = nc.dram_tensor(f"{p}_xgt2", [128, T * 128], bf16,
                                           kind="ExternalInput")
        dram[f"{p}_xg"] = nc.dram_tensor(f"{p}_xg", [128, T * 66], bf16,
                                         kind="ExternalInput")
        dram[f"{p}_cm"] = nc.dram_tensor(f"{p}_cm", [128, TC], bf16,
                                         kind="ExternalInput")
    dram["cstf"] = nc.dram_tensor("cstf", [128, 72], f32,
                                  kind="ExternalInput")
    dram["cstb"] = nc.dram_tensor("cstb", [128, 260], bf16,
                                  kind="ExternalInput")
    out_dram = nc.dram_tensor("out", [1, TC], f32, kind="ExternalOutput")

    with tile.TileContext(nc) as tc, ExitStack() as ctx:
        const = ctx.enter_context(tc.tile_pool(name="const", bufs=1))
        xp = ctx.enter_context(tc.tile_pool(name="xp", bufs=2))
        wk = ctx.enter_context(tc.tile_pool(name="wk", bufs=2))
        pse = ctx.enter_context(tc.tile_pool(name="pse", bufs=1,
                                             space="PSUM"))
        psy = ctx.enter_context(tc.tile_pool(name="psy", bufs=2,
                                             space="PSUM"))
        psr = ctx.enter_context(tc.tile_pool(name="psr", bufs=2,
                                             space="PSUM"))
        psh = ctx.enter_context(tc.tile_pool(name="psh", bufs=2,
                                             space="PSUM"))

        cstf = const.tile([128, 72], f32, tag="cstf")
        nc.gpsimd.dma_start(cstf[:], dram["cstf"].ap())
        cstb = const.tile([128, 260], bf16, tag="cstb")
        nc.gpsimd.dma_start(cstb[:], dram["cstb"].ap())
        eps = cstf[0:1, 4:5]
        mlpb = cstf[0:1, 5:6]
        biases = {"u": cstf[:, 6:7], "d": cstf[:, 7:8]}
        ones64 = cstf[0:1, 8:72]
        Ws = {"u": cstb[0:64, 0:128], "d": cstb[0:64, 128:256]}
        mlpW = cstb[:, 256:257]
        w2s = {"u": cstb[:, 257:258], "d": cstb[:, 258:259]}

        st = {"u": {}, "d": {}}
        # ---- DMA issue: u first on both HWDGE engines, then d ----
        for p, eng in (("u", nc.sync), ("d", nc.scalar)):
            s = st[p]
            s["xgt2"] = []
            s["xg"] = []
            for c in range(2):
                t = xp.tile([128, HT * 128], bf16, tag=f"xgt2_{c}",
                            name=f"xgt2_{c}_{p}")
                eng.dma_start(t[:], dram[f"{p}_xgt2"].ap()
                              [:, c * HT * 128:(c + 1) * HT * 128])
                s["xgt2"].append(t)
            for c in range(2):
                t = xp.tile([128, HT * 66], bf16, tag=f"xg_{c}",
                            name=f"xg_{c}_{p}")
                eng.dma_start(t[:], dram[f"{p}_xg"].ap()
                              [:, c * HT * 66:(c + 1) * HT * 66])
                s["xg"].append(t)
            t = xp.tile([128, TC], bf16, tag="cm", name=f"cm_{p}")
            eng.dma_start(t[:], dram[f"{p}_cm"].ap())
            s["cm"] = t

        # ---- per-slot logits -> P masks -> aggregation ----
        eraw = pse.tile([128, 2 * T], f32, tag="eraw")
        for p in ("u", "d"):
            s = st[p]
            off = 0 if p == "u" else T
            for c in range(2):
                xt = s["xgt2"][c]
                for t in range(HT):
                    nc.tensor.matmul(
                        eraw[:, off + c * HT + t: off + c * HT + t + 1],
                        xt[:, 128 * t:128 * t + 128],
                        w2s[p], start=True, stop=True)
            cmv = s["cm"][:].rearrange("p (t c) -> p t c", c=C)
            s["M"] = []
            for c in range(2):
                # exp(leaky_relu(x)) == max(exp(x), exp(0.2*x))
                e1 = wk.tile([128, HT], f32, tag="e1", name=f"e1_{p}{c}")
                nc.scalar.activation(
                    e1[:], eraw[:, off + c * HT: off + (c + 1) * HT],
                    Act.Exp)
                e2 = wk.tile([128, HT], f32, tag="e2", name=f"e2_{p}{c}")
                nc.scalar.activation(
                    e2[:], eraw[:, off + c * HT: off + (c + 1) * HT],
                    Act.Exp, scale=NEG)
                ex = wk.tile([128, HT], f32, tag="ex", name=f"ex_{p}{c}")
                nc.vector.tensor_tensor(ex[:], e1[:], e2[:], op=Alu.max)
                M = wk.tile([128, HT * C], bf16, tag="M", name=f"M_{p}{c}")
                Mv = M[:].rearrange("p (t c) -> p t c", c=C)
                for j in range(C):
                    nc.vector.tensor_tensor(
                        Mv[:, :, j], ex[:], cmv[:, c * HT:(c + 1) * HT, j],
                        op=Alu.mult)
                s["M"].append(M)

        for p in ("u", "d"):
            s = st[p]
            s["ynT"] = []
            for c in range(2):
                ynT = psy.tile([128, HT * C], f32, tag="ynT",
                               name=f"ynT_{p}{c}")
                xg = s["xg"][c]
                M = s["M"][c]
                for t in range(HT):
                    nc.tensor.matmul(
                        ynT[0:66, C * t:C * (t + 1)],
                        xg[:, 66 * t:66 * t + 66],
                        M[:, C * t:C * (t + 1)], start=True, stop=True)
                s["ynT"].append(ynT)

        # ---- tails: normalize, project, sigmoid ----
        sig = {}
        for p in ("u", "d"):
            s = st[p]
            ysb = wk.tile([65, TC], f32, tag="ysb", name=f"ysb_{p}")
            for c in range(2):
                nc.vector.tensor_copy(
                    ysb[:, c * HT * C:(c + 1) * HT * C],
                    s["ynT"][c][0:65, :])
            dn = wk.tile([1, TC], f32, tag="dn", name=f"dn_{p}")
            nc.vector.tensor_scalar(dn[:], ysb[64:65, :], eps, None,
                                    op0=Alu.add)
            rp = wk.tile([1, TC], f32, tag="rp", name=f"rp_{p}")
            nc.vector.reciprocal_approx_fast(rp[:], dn[:])
            ynrm = wk.tile([64, TC], bf16, tag="ynrm", name=f"ynrm_{p}")
            sg = wk.tile([128, TC], bf16, tag="sig", name=f"sig_{p}")
            sig[p] = sg
            for c in range(2):
                cs = slice(c * HT * C, (c + 1) * HT * C)
                rbc = psr.tile([64, HT * C], f32, tag="rbc",
                               name=f"rbc_{p}{c}")
                nc.tensor.matmul(rbc[:], ones64, rp[:, cs],
                                 start=True, stop=True)
                nc.vector.tensor_tensor(ynrm[:, cs], ysb[0:64, cs], rbc[:],
                                        op=Alu.mult)
                hT = psh.tile([128, HT * C], f32, tag="hT",
                              name=f"hT_{p}{c}")
                nc.tensor.matmul(hT[:], Ws[p], ynrm[:, cs],
                                 start=True, stop=True)
                nc.scalar.activation(sg[:, cs], hT[:], Act.Sigmoid,
                                     bias=biases[p])

        # ---- combine + MLP head ----
        prod = wk.tile([128, TC], bf16, tag="prod")
        nc.vector.tensor_tensor(prod[:], sig["u"][:], sig["d"][:],
                                op=Alu.mult)
        o_sb = wk.tile([1, TC], f32, tag="o_sb")
        for c in range(2):
            cs = slice(c * HT * C, (c + 1) * HT * C)
            o_ps = pse.tile([1, HT * C], f32, tag="mix", name=f"o{c}")
            nc.tensor.matmul(o_ps[:], mlpW, prod[:, cs],
                             start=True, stop=True)
            nc.vector.tensor_scalar(o_sb[:, cs], o_ps[:], mlpb, None,
                                    op0=Alu.add)
        nc.sync.dma_start(out_dram.ap(), o_sb[:])

    nc.compile()
    return nc


def _get_module():
    if "nc" not in _CACHE:
        _CACHE["nc"] = _build_module()
    return _CACHE["nc"]


def _pack_core(szs):
    """First-fit-decreasing pack of GPC graphs into <=T tiles of 128
    slots, <=C graphs each.  Returns (tile, col, offset) per graph."""
    order = np.argsort(-szs, kind="stable")
    used = []                       # [slots_used, ncols]
    gt = np.zeros(GPC, np.int32)
    gj = np.zeros(GPC, np.int32)
    go = np.zeros(GPC, np.int32)
    for g in order:
        s = int(szs[g])
        for ti in range(len(used)):
            if used[ti][0] + s <= 128 and used[ti][1] < C:
                break
        else:
            used.append([0, 0])
            ti = len(used) - 1
        gt[g] = ti
        gj[g] = used[ti][1]
        go[g] = used[ti][0]
        used[ti][0] += s
        used[ti][1] += 1
    assert len(used) <= T, f"pack needs {len(used)} tiles > {T}"
    return gt, gj, go


def _build_in_maps(inputs):
    import ml_dtypes
    bf = ml_dtypes.bfloat16

    data = {}
    sz = {}
    for p, q in (("u", "up"), ("d", "down")):
        x = np.asarray(inputs[f"{q}_x"], np.float32)
        ei = np.asarray(inputs[f"{q}_edge_index"]).astype(np.int64)
        src, dst = ei[0], ei[1]
        valid = (dst % NPG) == (NPG - 1)
        cnt = np.bincount(src[valid], minlength=N).astype(np.float32)
        W = np.asarray(inputs[f"{q}_W"], np.float32)
        w_src = W @ np.asarray(inputs[f"{q}_att_src"], np.float32)
        w_dst = W @ np.asarray(inputs[f"{q}_att_dst"], np.float32)
        data[p] = dict(x=x, cnt=cnt, W=W, w_src=w_src, w_dst=w_dst,
                       bias=np.asarray(inputs[f"{q}_bias"], np.float32))
        sz[p] = (cnt.reshape(G, NPG) > 0).sum(1)
    mx = np.maximum(sz["u"], sz["d"])

    pp = np.arange(128)
    cstf = np.zeros((128, 72), np.float32)
    cstf[0, 4] = 1e-16
    cstf[0, 5] = float(np.asarray(inputs["mlp_b"]).reshape(-1)[0])
    cstf[:, 6] = np.pad(data["u"]["bias"], (0, 0))
    cstf[:, 7] = data["d"]["bias"]
    cstf[0, 8:72] = 1.0
    cstb = np.zeros((128, 260), np.float32)
    cstb[0:64, 0:128] = data["u"]["W"]
    cstb[0:64, 128:256] = data["d"]["W"]
    cstb[:, 256] = np.asarray(inputs["mlp_W"], np.float32).reshape(H)
    cstb[0:64, 257] = data["u"]["w_src"]
    cstb[64:128, 257] = data["u"]["w_dst"]
    cstb[0:64, 258] = data["d"]["w_src"]
    cstb[64:128, 258] = data["d"]["w_dst"]
    common = {"cstf": cstf, "cstb": cstb.astype(bf)}

    in_maps = []
    colmaps = []
    for cidx in range(NC):
        g0 = cidx * GPC
        gt, gj, go = _pack_core(mx[g0:g0 + GPC])
        m = dict(common)
        for p in ("u", "d"):
            b = data[p]
            cnt_c = b["cnt"][g0 * NPG:(g0 + GPC) * NPG]
            nzl = np.nonzero(cnt_c > 0)[0]          # local node idx
            gl = nzl // NPG                          # local graph
            rank = np.arange(len(nzl)) - np.searchsorted(nzl // NPG, gl)
            mm = go[gl] + rank
            tt = gt[gl]
            xrows = b["x"][g0 * NPG + nzl]           # [nnz, 64]
            xlast = b["x"][(g0 + gl) * NPG + NPG - 1]
            xg = np.zeros((128, T, 66), np.float32)
            xg[:, :, 64] = 1.0
            xg[mm, tt, :64] = xrows
            xgt2 = np.zeros((128, T, 128), np.float32)
            xgt2[:64, tt, mm] = xrows.T
            xgt2[64:, tt, mm] = xlast.T
            cm = np.zeros((128, T, C), np.float32)
            cm[mm, tt, gj[gl]] = cnt_c[nzl]
            m[f"{p}_xg"] = np.ascontiguousarray(
                xg.reshape(128, T * 66)).astype(bf)
            m[f"{p}_xgt2"] = np.ascontiguousarray(
                xgt2.transpose(0, 1, 2).reshape(128, T * 128)).astype(bf)
            m[f"{p}_cm"] = np.ascontiguousarray(
                cm.reshape(128, TC)).astype(bf)
        in_maps.append(m)
        colmaps.append(gt.astype(np.int64) * C + gj)
    return in_maps, colmaps


def _gather_out(results, colmaps):
    outs = []
    for r, cmap in zip(results, colmaps):
        o = np.asarray(r["out"], np.float32).reshape(TC)
        outs.append(o[cmap])
    return np.concatenate(outs).reshape(G, 1)


def kernel(**inputs):
    from concourse.bass_utils import run_bass_kernel_spmd

    nc = _get_module()
    in_maps, colmaps = _build_in_maps(inputs)
    res = run_bass_kernel_spmd(nc, in_maps, core_ids=list(range(NC)))
    return _gather_out(res.results, colmaps)


# revision 4
# speedup vs baseline: 1.4823x; 1.1384x over previous
"""Trainium2 Bass kernel for nn_Event_Critic_Net (dual-branch GAT critic).

Math: the reference reads the GAT output only at the LAST node of each
graph (graphs are 32 contiguous nodes), so only edges with dst == last
node contribute.  Per graph g:

    out_g = sigmoid( (sum_n alpha[n] x[n]) @ W + bias )
    alpha[n] = cnt[n] exp(e[n]) / (sum + 1e-16)
    e[n] = leaky_relu(x[n].w_src + x[last(g)].w_dst)

Only nodes with cnt>0 matter (~7 of 32 per graph), so the host GATHERS
contributing nodes and bin-packs graphs into 128-slot tiles, T=36 tiles
x C=20 graph-columns per core per branch.  Per-slot logits come from a
single matmul per tile: the feature-major tensor stacks the slot's own
features (rows 0:64) on top of its graph's last-node features (rows
64:128), so lhsT.T @ [w_src; w_dst] yields e directly - no broadcast
chain.  exp(leaky(x)) = max(exp(x), exp(0.2x)) splits over two scalar
activations.  Aggregation contracts slots on the PE with a host-built
cnt-mask (one broadcast multiply per chunk).  Each branch chunk ships
as ONE concatenated DMA.  Graphs are data-parallel over 8 cores.
"""

import numpy as np
from contextlib import ExitStack

NC = 8            # cores
N = 131072        # nodes total
G = 4096          # graphs
NPG = 32          # nodes per graph
S = 64            # state size
H = 128           # hidden size
GPC = G // NC     # 512 graphs per core
T = 36            # slot tiles per branch per core
C = 20            # graph columns per tile
TC = T * C        # 720 output columns per core
HT = T // 2       # tiles per chunk (2 chunks)
NEG = 0.2

XGT2W = HT * 128  # 2304
XGW = HT * 66     # 1188
CMW = HT * C      # 360
CHW = XGT2W + XGW + CMW  # 3852 cols per chunk

_CACHE = {}


def _build_module():
    import concourse.tile as tile
    from concourse import bacc, mybir
    from concourse.alu_op_type import AluOpType as Alu

    f32 = mybir.dt.float32
    bf16 = mybir.dt.bfloat16
    Act = mybir.ActivationFunctionType

    nc = bacc.Bacc("TRN2", target_bir_lowering=False, debug=False,
                   num_devices=NC)

    dram = {}
    for p in ("u", "d"):
        dram[p] = nc.dram_tensor(f"{p}_dat", [128, 2 * CHW], bf16,
                                 kind="ExternalInput")
    dram["cstf"] = nc.dram_tensor("cstf", [128, 72], f32,
                                  kind="ExternalInput")
    dram["cstb"] = nc.dram_tensor("cstb", [128, 260], bf16,
                                  kind="ExternalInput")
    out_dram = nc.dram_tensor("out", [1, TC], f32, kind="ExternalOutput")

    with tile.TileContext(nc) as tc, ExitStack() as ctx:
        const = ctx.enter_context(tc.tile_pool(name="const", bufs=1))
        xp = ctx.enter_context(tc.tile_pool(name="xp", bufs=2))
        wk = ctx.enter_context(tc.tile_pool(name="wk", bufs=2))
        pse = ctx.enter_context(tc.tile_pool(name="pse", bufs=1,
                                             space="PSUM"))
        psy = ctx.enter_context(tc.tile_pool(name="psy", bufs=2,
                                             space="PSUM"))
        psr = ctx.enter_context(tc.tile_pool(name="psr", bufs=2,
                                             space="PSUM"))
        psh = ctx.enter_context(tc.tile_pool(name="psh", bufs=2,
                                             space="PSUM"))

        cstb = const.tile([128, 260], bf16, tag="cstb")
        nc.sync.dma_start(cstb[:], dram["cstb"].ap())
        cstf = const.tile([128, 72], f32, tag="cstf")
        nc.scalar.dma_start(cstf[:], dram["cstf"].ap())
        eps = cstf[0:1, 4:5]
        mlpb = cstf[0:1, 5:6]
        biases = {"u": cstf[:, 6:7], "d": cstf[:, 7:8]}
        ones64 = cstf[0:1, 8:72]
        Ws = {"u": cstb[0:64, 0:128], "d": cstb[0:64, 128:256]}
        mlpW = cstb[:, 256:257]
        w2s = {"u": cstb[:, 257:258], "d": cstb[:, 258:259]}

        # ---- loads: chunk c of branch p as one DMA; u first on both ----
        st = {"u": {}, "d": {}}
        for p in ("u", "d"):
            st[p]["ch"] = []
            for c, eng in ((0, nc.sync), (1, nc.scalar)):
                t = xp.tile([128, CHW], bf16, tag=f"ch{c}",
                            name=f"ch{c}_{p}")
                eng.dma_start(t[:], dram[p].ap()[:, c * CHW:(c + 1) * CHW])
                st[p]["ch"].append(t)

        # ---- per-slot logits -> P mask -> aggregation ----
        eraw = pse.tile([128, 2 * T], f32, tag="eraw")
        for p in ("u", "d"):
            s = st[p]
            off = 0 if p == "u" else T
            for c in range(2):
                ch = s["ch"][c]
                for t in range(HT):
                    nc.tensor.matmul(
                        eraw[:, off + c * HT + t: off + c * HT + t + 1],
                        ch[:, 128 * t:128 * t + 128],
                        w2s[p], start=True, stop=True)
            s["M"] = []
            for c in range(2):
                ch = s["ch"][c]
                er = eraw[:, off + c * HT: off + (c + 1) * HT]
                # exp(leaky_relu(x)) == max(exp(x), exp(0.2*x))
                e1 = wk.tile([128, HT], f32, tag="e1", name=f"e1_{p}{c}")
                nc.scalar.activation(e1[:], er, Act.Exp)
                e2 = wk.tile([128, HT], f32, tag="e2", name=f"e2_{p}{c}")
                nc.scalar.activation(e2[:], er, Act.Exp, scale=NEG)
                ex = wk.tile([128, HT], f32, tag="ex", name=f"ex_{p}{c}")
                nc.vector.tensor_tensor(ex[:], e1[:], e2[:], op=Alu.max)
                M = wk.tile([128, CMW], bf16, tag="M", name=f"M_{p}{c}")
                Mv = M[:].rearrange("p (t c) -> p t c", c=C)
                cmv = ch[:, XGT2W + XGW:CHW].rearrange(
                    "p (t c) -> p t c", c=C)
                exb = ex[:][:, :, None].broadcast_to([128, HT, C])
                nc.vector.tensor_tensor(Mv, exb, cmv, op=Alu.mult)
                s["M"].append(M)

            s["ynT"] = []
            for c in range(2):
                ynT = psy.tile([128, CMW], f32, tag="ynT",
                               name=f"ynT_{p}{c}")
                ch = s["ch"][c]
                M = s["M"][c]
                for t in range(HT):
                    nc.tensor.matmul(
                        ynT[0:66, C * t:C * (t + 1)],
                        ch[:, XGT2W + 66 * t:XGT2W + 66 * t + 66],
                        M[:, C * t:C * (t + 1)], start=True, stop=True)
                s["ynT"].append(ynT)

        # ---- tails: normalize, project, sigmoid ----
        sig = {}
        for p in ("u", "d"):
            s = st[p]
            ysb = wk.tile([65, TC], f32, tag="ysb", name=f"ysb_{p}")
            dn = wk.tile([1, TC], f32, tag="dn", name=f"dn_{p}")
            rp = wk.tile([1, TC], f32, tag="rp", name=f"rp_{p}")
            ynrm = wk.tile([64, TC], bf16, tag="ynrm", name=f"ynrm_{p}")
            sg = wk.tile([128, TC], bf16, tag="sig", name=f"sig_{p}")
            sig[p] = sg
            for c in range(2):
                cs = slice(c * CMW, (c + 1) * CMW)
                ynT = s["ynT"][c]
                nc.scalar.activation(ysb[:, cs], ynT[0:65, :], Act.Copy)
                nc.vector.tensor_scalar(dn[:, cs], ynT[64:65, :], eps,
                                        None, op0=Alu.add)
                nc.vector.reciprocal_approx_fast(rp[:, cs], dn[:, cs])
                rbc = psr.tile([64, CMW], f32, tag="rbc",
                               name=f"rbc_{p}{c}")
                nc.tensor.matmul(rbc[:], ones64, rp[:, cs],
                                 start=True, stop=True)
                nc.vector.tensor_tensor(ynrm[:, cs], ysb[0:64, cs], rbc[:],
                                        op=Alu.mult)
                hT = psh.tile([128, CMW], f32, tag="hT",
                              name=f"hT_{p}{c}")
                nc.tensor.matmul(hT[:], Ws[p], ynrm[:, cs],
                                 start=True, stop=True)
                nc.scalar.activation(sg[:, cs], hT[:], Act.Sigmoid,
                                     bias=biases[p])

        # ---- combine + MLP head ----
        prod = wk.tile([128, TC], bf16, tag="prod")
        o_sb = wk.tile([1, TC], f32, tag="o_sb")
        for c in range(2):
            cs = slice(c * CMW, (c + 1) * CMW)
            nc.vector.tensor_tensor(prod[:, cs], sig["u"][:, cs],
                                    sig["d"][:, cs], op=Alu.mult)
            o_ps = pse.tile([1, CMW], f32, tag="mix", name=f"o{c}")
            nc.tensor.matmul(o_ps[:], mlpW, prod[:, cs],
                             start=True, stop=True)
            nc.vector.tensor_scalar(o_sb[:, cs], o_ps[:], mlpb, None,
                                    op0=Alu.add)
        nc.sync.dma_start(out_dram.ap(), o_sb[:])

    nc.compile()
    return nc


def _get_module():
    if "nc" not in _CACHE:
        _CACHE["nc"] = _build_module()
    return _CACHE["nc"]


def _pack_core(szs):
    """First-fit-decreasing pack of GPC graphs into <=T tiles of 128
    slots, <=C graphs each.  Returns (tile, col, offset) per graph."""
    order = np.argsort(-szs, kind="stable")
    used = []                       # [slots_used, ncols]
    gt = np.zeros(GPC, np.int32)
    gj = np.zeros(GPC, np.int32)
    go = np.zeros(GPC, np.int32)
    for g in order:
        s = int(szs[g])
        for ti in range(len(used)):
            if used[ti][0] + s <= 128 and used[ti][1] < C:
                break
        else:
            used.append([0, 0])
            ti = len(used) - 1
        gt[g] = ti
        gj[g] = used[ti][1]
        go[g] = used[ti][0]
        used[ti][0] += s
        used[ti][1] += 1
    assert len(used) <= T, f"pack needs {len(used)} tiles > {T}"
    return gt, gj, go


def _build_in_maps(inputs):
    import ml_dtypes
    bf = ml_dtypes.bfloat16

    data = {}
    sz = {}
    for p, q in (("u", "up"), ("d", "down")):
        x = np.asarray(inputs[f"{q}_x"], np.float32)
        ei = np.asarray(inputs[f"{q}_edge_index"]).astype(np.int64)
        src, dst = ei[0], ei[1]
        valid = (dst % NPG) == (NPG - 1)
        cnt = np.bincount(src[valid], minlength=N).astype(np.float32)
        W = np.asarray(inputs[f"{q}_W"], np.float32)
        w_src = W @ np.asarray(inputs[f"{q}_att_src"], np.float32)
        w_dst = W @ np.asarray(inputs[f"{q}_att_dst"], np.float32)
        data[p] = dict(x=x, cnt=cnt, W=W, w_src=w_src, w_dst=w_dst,
                       bias=np.asarray(inputs[f"{q}_bias"], np.float32))
        sz[p] = (cnt.reshape(G, NPG) > 0).sum(1)
    mx = np.maximum(sz["u"], sz["d"])

    cstf = np.zeros((128, 72), np.float32)
    cstf[0, 4] = 1e-16
    cstf[0, 5] = float(np.asarray(inputs["mlp_b"]).reshape(-1)[0])
    cstf[:, 6] = data["u"]["bias"]
    cstf[:, 7] = data["d"]["bias"]
    cstf[0, 8:72] = 1.0
    cstb = np.zeros((128, 260), np.float32)
    cstb[0:64, 0:128] = data["u"]["W"]
    cstb[0:64, 128:256] = data["d"]["W"]
    cstb[:, 256] = np.asarray(inputs["mlp_W"], np.float32).reshape(H)
    cstb[0:64, 257] = data["u"]["w_src"]
    cstb[64:128, 257] = data["u"]["w_dst"]
    cstb[0:64, 258] = data["d"]["w_src"]
    cstb[64:128, 258] = data["d"]["w_dst"]
    common = {"cstf": cstf, "cstb": cstb.astype(bf)}

    in_maps = []
    colmaps = []
    for cidx in range(NC):
        g0 = cidx * GPC
        gt, gj, go = _pack_core(mx[g0:g0 + GPC])
        m = dict(common)
        for p in ("u", "d"):
            b = data[p]
            cnt_c = b["cnt"][g0 * NPG:(g0 + GPC) * NPG]
            nzl = np.nonzero(cnt_c > 0)[0]          # local node idx
            gl = nzl // NPG                          # local graph
            rank = np.arange(len(nzl)) - np.searchsorted(nzl // NPG, gl)
            mm = go[gl] + rank
            tt = gt[gl]
            xrows = b["x"][g0 * NPG + nzl]           # [nnz, 64]
            xlast = b["x"][(g0 + gl) * NPG + NPG - 1]
            xg = np.zeros((128, T, 66), np.float32)
            xg[:, :, 64] = 1.0
            xg[mm, tt, :64] = xrows
            xgt2 = np.zeros((128, T, 128), np.float32)
            xgt2[:64, tt, mm] = xrows.T
            xgt2[64:, tt, mm] = xlast.T
            cm = np.zeros((128, T, C), np.float32)
            cm[mm, tt, gj[gl]] = cnt_c[nzl]
            chunks = []
            for c in range(2):
                ts = slice(c * HT, (c + 1) * HT)
                chunks.append(np.concatenate([
                    xgt2[:, ts].reshape(128, XGT2W),
                    xg[:, ts].reshape(128, XGW),
                    cm[:, ts].reshape(128, CMW)], axis=1))
            m[f"{p}_dat"] = np.ascontiguousarray(
                np.concatenate(chunks, axis=1)).astype(bf)
        in_maps.append(m)
        colmaps.append(gt.astype(np.int64) * C + gj)
    return in_maps, colmaps


def _gather_out(results, colmaps):
    outs = []
    for r, cmap in zip(results, colmaps):
        o = np.asarray(r["out"], np.float32).reshape(TC)
        outs.append(o[cmap])
    return np.concatenate(outs).reshape(G, 1)


def kernel(**inputs):
    from concourse.bass_utils import run_bass_kernel_spmd

    nc = _get_module()
    in_maps, colmaps = _build_in_maps(inputs)
    res = run_bass_kernel_spmd(nc, in_maps, core_ids=list(range(NC)))
    return _gather_out(res.results, colmaps)
